# revision 8
# baseline (speedup 1.0000x reference)
"""Bahdanau attention Trainium2 kernel (8 NeuronCores, data-parallel over batch).

reference math (per batch b):
    proj_f = features[b] @ W1 + b1            # [L, U]
    proj_h = hidden[b] @ W2 + b2              # [U]
    score  = tanh(proj_f + proj_h)            # [L, U]
    logits = score @ V + bV                   # [L, 1]
    attn   = softmax(logits, axis=0)          # [L, 1]
    ctx    = sum_l attn[l] * features[b, l]   # [D]

Notes:
  - bV shifts all logits equally -> softmax-invariant -> dropped.
  - softmax computed without max subtraction: |logits| <= sum|V| ~ 13, exp is
    safe in f32.
  - compute dtype fp16 (quantization ~5e-4 rel err), f32 accumulation in PSUM.
  - L is processed under the permutation l = p*16 + c (p: partition, c: chunk
    col). All reductions over l are order-invariant, so only the DMA access
    patterns encode the permutation; it makes both the feature load and the
    attention-weights store contiguous per partition.

Layouts per core (16 batches):
  f_nat  [128, 16*256] fp16   f_nat[p, c*256+d] = f[p*16+c, d]  (16KB/partition
                              contiguous DMA-cast load)
  fT     [128, 2, 2048] fp16  fT[p, dh, c*128+q] = f[q*16+c, dh*128+p]
  scoreT [128(u-half), 2048] fp16 x2  (W1-stationary proj + ACT tanh,
                              per-partition bias = hprojT[:, b])
  logits [128, 16] psum; e [128, 16]; weights out = e*(1/Z) stored as [128, 16]
  context via PE matmuls with e columns as lhsT over f_nat chunks.
"""

import os
import numpy as np

import concourse.bass as bass
import concourse.tile as tile
from concourse import bacc
from concourse import mybir
from concourse.bass_utils import run_bass_kernel_spmd
from concourse.masks import make_identity

B, L, D = 128, 2048, 256
H, U = 512, 256
NCORES = 8
BPC = B // NCORES  # batches per core
NC_ = L // 128  # 16 L-chunks per batch
F16 = mybir.dt.float16
F32 = mybir.dt.float32

_CACHE = {}


def _build():
    nc = bacc.Bacc("TRN2", target_bir_lowering=False, debug=False)

    feats = nc.declare_dram_parameter("features", [BPC, L, D], F32, isOutput=False)
    hid = nc.declare_dram_parameter("hidden", [BPC, H], F32, isOutput=False)
    # host pre-laid-out weights: w1p [128, 2*U] (k-major), w2p [128, 4*U],
    # vp [128, 2], b12 [1, U] (= b1 + b2)
    w1p = nc.declare_dram_parameter("w1p", [128, 2 * U], F32, isOutput=False)
    w2p = nc.declare_dram_parameter("w2p", [128, 4 * U], F32, isOutput=False)
    vp = nc.declare_dram_parameter("vp", [128, 2], F32, isOutput=False)
    b12 = nc.declare_dram_parameter("b12", [1, U], F32, isOutput=False)

    ctx_out = nc.declare_dram_parameter("ctx", [BPC, D], F32, isOutput=True)
    # attn stored per batch as [128, 16]: attn[b, p*16 + c] = w_sb[p, c]
    attn_out = nc.declare_dram_parameter("attn", [BPC, L], F32, isOutput=True)

    with tile.TileContext(nc) as tc:
        with (
            tc.tile_pool(name="const", bufs=1) as const,
            tc.tile_pool(name="fpool", bufs=8) as fpool,
            tc.tile_pool(name="ftpool", bufs=3) as ftpool,
            tc.tile_pool(name="spool", bufs=4) as spool,
            tc.tile_pool(name="epool", bufs=3) as epool,
            tc.tile_pool(name="opool", bufs=3) as opool,
            tc.tile_pool(name="pt", bufs=2, space="PSUM") as pt_pool,
            tc.tile_pool(name="pj", bufs=2, space="PSUM") as pj_pool,
            tc.tile_pool(name="ps", bufs=2, space="PSUM") as ps_pool,
        ):
            # ---------------- setup ----------------
            # queue the first feature loads before anything else so the PE
            # pipeline can start as soon as possible
            fv_all = [
                feats[b].rearrange("(p c) d -> p (c d)", p=128) for b in range(BPC)
            ]
            half = NC_ * D // 2
            f_nats = {}
            for b in range(2):
                fn = fpool.tile([128, NC_ * D], F16, tag="fnat", name=f"fnat{b}")
                nc.gpsimd.dma_start(fn[:, 0:half], fv_all[b][:, 0:half])
                nc.gpsimd.dma_start(fn[:, half:], fv_all[b][:, half:])
                f_nats[b] = fn

            ident = const.tile([128, 128], F16)
            make_identity(nc, ident)
            ones_sq = const.tile([128, 128], F16)
            nc.gpsimd.memset(ones_sq, 1.0)
            ones_row = const.tile([1, 128], F16)
            nc.gpsimd.memset(ones_row, 1.0)

            w1_sb = const.tile([128, 2 * U], F16)
            nc.gpsimd.dma_start(w1_sb[:, :], w1p[:, :])
            w2_sb = const.tile([128, 4 * U], F16)
            nc.gpsimd.dma_start(w2_sb[:, :], w2p[:, :])
            v_sb = const.tile([128, 2], F16)
            nc.gpsimd.dma_start(v_sb[:, :], vp[:, :])
            b12_sb = const.tile([1, U], F16)
            nc.gpsimd.dma_start(b12_sb[:, :], b12[:, :])
            hid_sb = const.tile([BPC, H], F16)
            nc.gpsimd.dma_start(hid_sb[:, :], hid[:, :])

            # hidden transpose: hT[p, k*BPC + b] = hidden[b, k*128+p]
            hT = const.tile([128, 4 * BPC], F16)
            for k in range(4):
                pht = pt_pool.tile([128, 1024], F16, tag="ptr", name="pht")
                nc.tensor.transpose(
                    pht[:, 0:BPC],
                    hid_sb[:, k * 128 : (k + 1) * 128],
                    ident[0:BPC, 0:BPC],
                )
                nc.vector.tensor_copy(hT[:, k * BPC : (k + 1) * BPC], pht[:, 0:BPC])

            # hprojT[u, b] = sum_k W2[k, u] * h[b, k] + b1[u] + b2[u]
            hprojT = const.tile([128, 2 * BPC], F32)
            for uh in range(2):
                php = ps_pool.tile([128, BPC], F32, tag="small", name="php")
                for k in range(4):
                    nc.tensor.matmul(
                        php[:, :],
                        w2_sb[:, k * U + uh * 128 : k * U + uh * 128 + 128],
                        hT[:, k * BPC : (k + 1) * BPC],
                        start=(k == 0),
                        stop=False,
                    )
                nc.tensor.matmul(
                    php[:, :],
                    b12_sb[:, uh * 128 : (uh + 1) * 128],
                    ones_row[:, 0:BPC],
                    start=False,
                    stop=True,
                )
                nc.vector.tensor_copy(hprojT[:, uh * BPC : (uh + 1) * BPC], php[:, :])

            # ---------------- main loop over batches ----------------
            for b in range(BPC):
                # 1. load + cast f32 -> fp16, 16KB contiguous per partition:
                #    f_nat[p, c*256+d] = f[p*16+c, d]
                if b in f_nats:
                    f_nat = f_nats[b]
                else:
                    f_nat = fpool.tile([128, NC_ * D], F16, tag="fnat")
                    nc.gpsimd.dma_start(f_nat[:, 0:half], fv_all[b][:, 0:half])
                    nc.gpsimd.dma_start(f_nat[:, half:], fv_all[b][:, half:])

                # 2. transposes: fT[p, dh, c*128+q] = f_nat[q, c*256+dh*128+p]
                fT = ftpool.tile([128, 2 * L], F16, tag="ft")
                for g in range(2):  # groups of 8 chunks -> one psum bank each
                    for dh in range(2):
                        ptr = pt_pool.tile([128, 1024], F16, tag="ptr")
                        for j in range(8):
                            c = g * 8 + j
                            nc.tensor.transpose(
                                ptr[:, j * 128 : (j + 1) * 128],
                                f_nat[:, c * D + dh * 128 : c * D + dh * 128 + 128],
                                ident[:, :],
                            )
                        nc.vector.tensor_copy(
                            fT[:, dh * L + g * 1024 : dh * L + g * 1024 + 1024],
                            ptr[:, :],
                        )

                # 3-4. proj (W1-stationary, dh outer to halve reloads) + tanh
                score = [None, None]
                for uh in range(2):
                    score[uh] = spool.tile(
                        [128, L], F16, tag=f"score{uh}", name=f"score{uh}"
                    )
                    pjs = []
                    for g in range(2):
                        pjg = pj_pool.tile([128, 1024], F32, tag="pj", name="pj")
                        pjs.append(pjg)
                    for dh in range(2):  # dh outer: W1 tile loaded once per dh
                        for g in range(2):
                            for j in range(2):  # 512-wide matmuls
                                base = g * 1024 + j * 512
                                nc.tensor.matmul(
                                    pjs[g][:, j * 512 : (j + 1) * 512],
                                    w1_sb[
                                        :, dh * U + uh * 128 : dh * U + uh * 128 + 128
                                    ],
                                    fT[:, dh * L + base : dh * L + base + 512],
                                    start=(dh == 0),
                                    stop=(dh == 1),
                                )
                    for g in range(2):
                        nc.scalar.activation(
                            score[uh][:, g * 1024 : (g + 1) * 1024],
                            pjs[g][:, :],
                            mybir.ActivationFunctionType.Tanh,
                            bias=hprojT[:, uh * BPC + b : uh * BPC + b + 1],
                        )

                # 5. logits[p, c] = sum_u V[u] * scoreT[u, c*128+p]
                plg = ps_pool.tile([128, NC_], F32, tag="small", name="plg")
                for c in range(NC_):
                    for uh in range(2):
                        nc.tensor.matmul(
                            plg[:, c : c + 1],
                            score[uh][:, c * 128 : (c + 1) * 128],
                            v_sb[:, uh : uh + 1],
                            start=(uh == 0),
                            stop=(uh == 1),
                        )

                # 6. e = exp(logits)  (no max subtraction; bounded)
                e32 = epool.tile([128, NC_], F32, tag="e32")
                nc.scalar.activation(
                    e32[:, :], plg[:, :], mybir.ActivationFunctionType.Exp
                )
                e16 = epool.tile([128, NC_], F16, tag="e16")
                nc.vector.tensor_copy(e16[:, :], e32[:, :])

                # 7. Z broadcast to all partitions via all-ones matmul
                zsum32 = epool.tile([128, 1], F32, tag="zsum32")
                nc.vector.reduce_sum(zsum32[:, :], e32[:, :], axis=mybir.AxisListType.X)
                zsum = epool.tile([128, 1], F16, tag="zsum")
                nc.vector.tensor_copy(zsum[:, :], zsum32[:, :])
                pz = ps_pool.tile([128, 1], F32, tag="small", name="pz")
                nc.tensor.matmul(
                    pz[:, :], ones_sq[:, :], zsum[:, :], start=True, stop=True
                )
                rz = epool.tile([128, 1], F32, tag="rz")
                nc.vector.reciprocal(rz[:, :], pz[:, :])

                # 8. context = (1/Z) * sum_c e[:, c].T @ f_nat[:, c, :]
                pcx = ps_pool.tile([1, D], F32, tag="small", name="pcx")
                for c in range(NC_):
                    nc.tensor.matmul(
                        pcx[:, :],
                        e16[:, c : c + 1],
                        f_nat[:, c * D : (c + 1) * D],
                        start=(c == 0),
                        stop=(c == NC_ - 1),
                    )
                ctx_sb = opool.tile([1, D], F32, tag="ctxsb")
                nc.vector.tensor_scalar_mul(ctx_sb[:, :], pcx[:, :], rz[0:1, :])
                nc.sync.dma_start(ctx_out[b : b + 1, :], ctx_sb[:, :])

                # 9. weights out: w[p, c] = e[p, c]/Z -> attn[b, p*16+c]
                w_sb = opool.tile([128, NC_], F32, tag="wsb")
                nc.vector.tensor_scalar_mul(w_sb[:, :], e32[:, :], rz[:, :])
                nc.sync.dma_start(
                    attn_out[b].rearrange("(p c) -> p c", p=128), w_sb[:, :]
                )

    nc.compile()
    return nc


def _prep_host(inputs):
    f32 = np.float32
    features = np.ascontiguousarray(np.asarray(inputs["features"], dtype=f32))
    hidden = np.ascontiguousarray(np.asarray(inputs["hidden"], dtype=f32))
    W1 = np.asarray(inputs["W1"], dtype=f32)
    W2 = np.asarray(inputs["W2"], dtype=f32)
    V = np.asarray(inputs["V"], dtype=f32)
    b1 = np.asarray(inputs["b1"], dtype=f32)
    b2 = np.asarray(inputs["b2"], dtype=f32)

    w1p = np.ascontiguousarray(
        W1.reshape(2, 128, U).transpose(1, 0, 2).reshape(128, 2 * U)
    )
    w2p = np.ascontiguousarray(
        W2.reshape(4, 128, U).transpose(1, 0, 2).reshape(128, 4 * U)
    )
    vp = np.ascontiguousarray(V.reshape(2, 128).T)
    b12 = np.ascontiguousarray((b1 + b2).reshape(1, U))

    in_maps = []
    for i in range(NCORES):
        in_maps.append(
            {
                "features": features[i * BPC : (i + 1) * BPC],
                "hidden": hidden[i * BPC : (i + 1) * BPC],
                "w1p": w1p,
                "w2p": w2p,
                "vp": vp,
                "b12": b12,
            }
        )
    return in_maps


def _run(inputs, trace=False):
    if "nc" not in _CACHE:
        _CACHE["nc"] = _build()
    nc = _CACHE["nc"]
    in_maps = _prep_host(inputs)
    res = run_bass_kernel_spmd(nc, in_maps, core_ids=list(range(NCORES)), trace=trace)
    ctx = np.concatenate([np.asarray(r["ctx"]) for r in res.results], axis=0)
    attn = np.concatenate([np.asarray(r["attn"]) for r in res.results], axis=0)
    return (ctx.astype(np.float32), attn.reshape(B, L, 1).astype(np.float32)), res


def kernel(**inputs):
    outs, _ = _run(inputs, trace=False)
    return outs


# revision 9
# speedup vs baseline: 1.0475x; 1.0475x over previous
"""Bahdanau attention Trainium2 kernel (8 NeuronCores, data-parallel over batch).

reference math (per batch b):
    proj_f = features[b] @ W1 + b1            # [L, U]
    proj_h = hidden[b] @ W2 + b2              # [U]
    score  = tanh(proj_f + proj_h)            # [L, U]
    logits = score @ V + bV                   # [L, 1]
    attn   = softmax(logits, axis=0)          # [L, 1]
    ctx    = sum_l attn[l] * features[b, l]   # [D]

Notes:
  - bV shifts all logits equally -> softmax-invariant -> dropped.
  - softmax computed without max subtraction: |logits| <= sum|V| ~ 13, exp is
    safe in f32.
  - compute dtype fp16 (quantization ~5e-4 rel err), f32 accumulation in PSUM.
  - L is processed under the permutation l = p*16 + c (p: partition, c: chunk
    col). All reductions over l are order-invariant, so only the DMA access
    patterns encode the permutation; it makes both the feature load and the
    attention-weights store contiguous per partition.

Layouts per core (16 batches):
  f_nat  [128, 16*256] fp16   f_nat[p, c*256+d] = f[p*16+c, d]  (16KB/partition
                              contiguous DMA-cast load)
  fT     [128, 2, 2048] fp16  fT[p, dh, c*128+q] = f[q*16+c, dh*128+p]
  scoreT [128(u-half), 2048] fp16 x2  (W1-stationary proj + ACT tanh,
                              per-partition bias = hprojT[:, b])
  logits [128, 16] psum; e [128, 16]; weights out = e*(1/Z) stored as [128, 16]
  context via PE matmuls with e columns as lhsT over f_nat chunks.
"""

import os
import numpy as np

import concourse.bass as bass
import concourse.tile as tile
from concourse import bacc
from concourse import mybir
from concourse.bass_utils import run_bass_kernel_spmd
from concourse.masks import make_identity

B, L, D = 128, 2048, 256
H, U = 512, 256
NCORES = 8
BPC = B // NCORES  # batches per core
NC_ = L // 128  # 16 L-chunks per batch
F16 = mybir.dt.float16
F32 = mybir.dt.float32

_CACHE = {}


def _build():
    nc = bacc.Bacc("TRN2", target_bir_lowering=False, debug=False)

    feats = nc.declare_dram_parameter("features", [BPC, L, D], F32, isOutput=False)
    hid = nc.declare_dram_parameter("hidden", [BPC, H], F32, isOutput=False)
    # host pre-laid-out weights: w1p [128, 2*U] (k-major), w2p [128, 4*U],
    # vp [128, 2], b12 [1, U] (= b1 + b2)
    w1p = nc.declare_dram_parameter("w1p", [128, 2 * U], F32, isOutput=False)
    w2p = nc.declare_dram_parameter("w2p", [128, 4 * U], F32, isOutput=False)
    vp = nc.declare_dram_parameter("vp", [128, 2], F32, isOutput=False)
    b12 = nc.declare_dram_parameter("b12", [1, U], F32, isOutput=False)

    ctx_out = nc.declare_dram_parameter("ctx", [BPC, D], F32, isOutput=True)
    # attn stored per batch as [128, 16]: attn[b, p*16 + c] = w_sb[p, c]
    attn_out = nc.declare_dram_parameter("attn", [BPC, L], F32, isOutput=True)

    with tile.TileContext(nc) as tc:
        with (
            tc.tile_pool(name="const", bufs=1) as const,
            tc.tile_pool(name="fpool", bufs=8) as fpool,
            tc.tile_pool(name="ftpool", bufs=3) as ftpool,
            tc.tile_pool(name="spool", bufs=4) as spool,
            tc.tile_pool(name="epool", bufs=3) as epool,
            tc.tile_pool(name="opool", bufs=3) as opool,
            tc.tile_pool(name="pt", bufs=2, space="PSUM") as pt_pool,
            tc.tile_pool(name="pj", bufs=2, space="PSUM") as pj_pool,
            tc.tile_pool(name="ps", bufs=2, space="PSUM") as ps_pool,
        ):
            # ---------------- setup ----------------
            # order: tiny weight DMAs first (finish fast, unblock hproj),
            # then feature prefetch for b=0/1, then gpsimd memsets.
            w1_sb = const.tile([128, 2 * U], F16)
            nc.gpsimd.dma_start(w1_sb[:, :], w1p[:, :])
            w2_sb = const.tile([128, 4 * U], F16)
            nc.gpsimd.dma_start(w2_sb[:, :], w2p[:, :])
            v_sb = const.tile([128, 2], F16)
            nc.gpsimd.dma_start(v_sb[:, :], vp[:, :])
            b12_sb = const.tile([1, U], F16)
            nc.gpsimd.dma_start(b12_sb[:, :], b12[:, :])
            hid_sb = const.tile([BPC, H], F16)
            nc.gpsimd.dma_start(hid_sb[:, :], hid[:, :])

            fv_all = [
                feats[b].rearrange("(p c) d -> p (c d)", p=128) for b in range(BPC)
            ]
            half = NC_ * D // 2
            f_nats = {}
            for b in range(2):
                fn = fpool.tile([128, NC_ * D], F16, tag="fnat", name=f"fnat{b}")
                nc.gpsimd.dma_start(fn[:, 0:half], fv_all[b][:, 0:half])
                nc.gpsimd.dma_start(fn[:, half:], fv_all[b][:, half:])
                f_nats[b] = fn

            ident = const.tile([128, 128], F16)
            make_identity(nc, ident)
            ones_sq = const.tile([128, 128], F16)
            nc.gpsimd.memset(ones_sq, 1.0)
            ones_row = const.tile([1, 128], F16)
            nc.gpsimd.memset(ones_row, 1.0)

            # hidden transpose: hT[p, k*BPC + b] = hidden[b, k*128+p]
            hT = const.tile([128, 4 * BPC], F16)
            for k in range(4):
                pht = pt_pool.tile([128, 1024], F16, tag="ptr", name="pht")
                nc.tensor.transpose(
                    pht[:, 0:BPC],
                    hid_sb[:, k * 128 : (k + 1) * 128],
                    ident[0:BPC, 0:BPC],
                )
                nc.vector.tensor_copy(hT[:, k * BPC : (k + 1) * BPC], pht[:, 0:BPC])

            # hprojT[u, b] = sum_k W2[k, u] * h[b, k] + b1[u] + b2[u]
            hprojT = const.tile([128, 2 * BPC], F32)
            for uh in range(2):
                php = ps_pool.tile([128, BPC], F32, tag="small", name="php")
                for k in range(4):
                    nc.tensor.matmul(
                        php[:, :],
                        w2_sb[:, k * U + uh * 128 : k * U + uh * 128 + 128],
                        hT[:, k * BPC : (k + 1) * BPC],
                        start=(k == 0),
                        stop=False,
                    )
                nc.tensor.matmul(
                    php[:, :],
                    b12_sb[:, uh * 128 : (uh + 1) * 128],
                    ones_row[:, 0:BPC],
                    start=False,
                    stop=True,
                )
                nc.vector.tensor_copy(hprojT[:, uh * BPC : (uh + 1) * BPC], php[:, :])

            # ---------------- main loop over batches ----------------
            for b in range(BPC):
                # 1. load + cast f32 -> fp16, 16KB contiguous per partition:
                #    f_nat[p, c*256+d] = f[p*16+c, d]
                if b in f_nats:
                    f_nat = f_nats[b]
                else:
                    f_nat = fpool.tile([128, NC_ * D], F16, tag="fnat")
                    nc.gpsimd.dma_start(f_nat[:, 0:half], fv_all[b][:, 0:half])
                    nc.gpsimd.dma_start(f_nat[:, half:], fv_all[b][:, half:])

                # 2. transposes: fT[p, dh, c*128+q] = f_nat[q, c*256+dh*128+p]
                fT = ftpool.tile([128, 2 * L], F16, tag="ft")
                for g in range(2):  # groups of 8 chunks -> one psum bank each
                    for dh in range(2):
                        ptr = pt_pool.tile([128, 1024], F16, tag="ptr")
                        for j in range(8):
                            c = g * 8 + j
                            nc.tensor.transpose(
                                ptr[:, j * 128 : (j + 1) * 128],
                                f_nat[:, c * D + dh * 128 : c * D + dh * 128 + 128],
                                ident[:, :],
                            )
                        nc.vector.tensor_copy(
                            fT[:, dh * L + g * 1024 : dh * L + g * 1024 + 1024],
                            ptr[:, :],
                        )

                # 3-4. proj (W1-stationary, dh outer to halve reloads) + tanh
                score = [None, None]
                for uh in range(2):
                    score[uh] = spool.tile(
                        [128, L], F16, tag=f"score{uh}", name=f"score{uh}"
                    )
                    pjs = []
                    for g in range(2):
                        pjg = pj_pool.tile([128, 1024], F32, tag="pj", name="pj")
                        pjs.append(pjg)
                    for dh in range(2):  # dh outer: W1 tile loaded once per dh
                        for g in range(2):
                            for j in range(2):  # 512-wide matmuls
                                base = g * 1024 + j * 512
                                nc.tensor.matmul(
                                    pjs[g][:, j * 512 : (j + 1) * 512],
                                    w1_sb[
                                        :, dh * U + uh * 128 : dh * U + uh * 128 + 128
                                    ],
                                    fT[:, dh * L + base : dh * L + base + 512],
                                    start=(dh == 0),
                                    stop=(dh == 1),
                                )
                    for g in range(2):
                        nc.scalar.activation(
                            score[uh][:, g * 1024 : (g + 1) * 1024],
                            pjs[g][:, :],
                            mybir.ActivationFunctionType.Tanh,
                            bias=hprojT[:, uh * BPC + b : uh * BPC + b + 1],
                        )

                # 5. logits[p, c] = sum_u V[u] * scoreT[u, c*128+p]
                plg = ps_pool.tile([128, NC_], F32, tag="small", name="plg")
                for c in range(NC_):
                    for uh in range(2):
                        nc.tensor.matmul(
                            plg[:, c : c + 1],
                            score[uh][:, c * 128 : (c + 1) * 128],
                            v_sb[:, uh : uh + 1],
                            start=(uh == 0),
                            stop=(uh == 1),
                        )

                # 6. e = exp(logits)  (no max subtraction; bounded)
                e32 = epool.tile([128, NC_], F32, tag="e32")
                nc.scalar.activation(
                    e32[:, :], plg[:, :], mybir.ActivationFunctionType.Exp
                )
                e16 = epool.tile([128, NC_], F16, tag="e16")
                nc.vector.tensor_copy(e16[:, :], e32[:, :])

                # 7. Z broadcast to all partitions via all-ones matmul
                zsum32 = epool.tile([128, 1], F32, tag="zsum32")
                nc.vector.reduce_sum(zsum32[:, :], e32[:, :], axis=mybir.AxisListType.X)
                zsum = epool.tile([128, 1], F16, tag="zsum")
                nc.vector.tensor_copy(zsum[:, :], zsum32[:, :])
                pz = ps_pool.tile([128, 1], F32, tag="small", name="pz")
                nc.tensor.matmul(
                    pz[:, :], ones_sq[:, :], zsum[:, :], start=True, stop=True
                )
                rz = epool.tile([128, 1], F32, tag="rz")
                nc.vector.reciprocal(rz[:, :], pz[:, :])

                # 8. context = (1/Z) * sum_c e[:, c].T @ f_nat[:, c, :]
                pcx = ps_pool.tile([1, D], F32, tag="small", name="pcx")
                for c in range(NC_):
                    nc.tensor.matmul(
                        pcx[:, :],
                        e16[:, c : c + 1],
                        f_nat[:, c * D : (c + 1) * D],
                        start=(c == 0),
                        stop=(c == NC_ - 1),
                    )
                ctx_sb = opool.tile([1, D], F32, tag="ctxsb")
                nc.vector.tensor_scalar_mul(ctx_sb[:, :], pcx[:, :], rz[0:1, :])
                nc.sync.dma_start(ctx_out[b : b + 1, :], ctx_sb[:, :])

                # 9. weights out: w[p, c] = e[p, c]/Z -> attn[b, p*16+c]
                w_sb = opool.tile([128, NC_], F32, tag="wsb")
                nc.vector.tensor_scalar_mul(w_sb[:, :], e32[:, :], rz[:, :])
                nc.sync.dma_start(
                    attn_out[b].rearrange("(p c) -> p c", p=128), w_sb[:, :]
                )

    nc.compile()
    return nc


def _prep_host(inputs):
    f32 = np.float32
    features = np.ascontiguousarray(np.asarray(inputs["features"], dtype=f32))
    hidden = np.ascontiguousarray(np.asarray(inputs["hidden"], dtype=f32))
    W1 = np.asarray(inputs["W1"], dtype=f32)
    W2 = np.asarray(inputs["W2"], dtype=f32)
    V = np.asarray(inputs["V"], dtype=f32)
    b1 = np.asarray(inputs["b1"], dtype=f32)
    b2 = np.asarray(inputs["b2"], dtype=f32)

    w1p = np.ascontiguousarray(
        W1.reshape(2, 128, U).transpose(1, 0, 2).reshape(128, 2 * U)
    )
    w2p = np.ascontiguousarray(
        W2.reshape(4, 128, U).transpose(1, 0, 2).reshape(128, 4 * U)
    )
    vp = np.ascontiguousarray(V.reshape(2, 128).T)
    b12 = np.ascontiguousarray((b1 + b2).reshape(1, U))

    in_maps = []
    for i in range(NCORES):
        in_maps.append(
            {
                "features": features[i * BPC : (i + 1) * BPC],
                "hidden": hidden[i * BPC : (i + 1) * BPC],
                "w1p": w1p,
                "w2p": w2p,
                "vp": vp,
                "b12": b12,
            }
        )
    return in_maps


def _run(inputs, trace=False):
    if "nc" not in _CACHE:
        _CACHE["nc"] = _build()
    nc = _CACHE["nc"]
    in_maps = _prep_host(inputs)
    res = run_bass_kernel_spmd(nc, in_maps, core_ids=list(range(NCORES)), trace=trace)
    ctx = np.concatenate([np.asarray(r["ctx"]) for r in res.results], axis=0)
    attn = np.concatenate([np.asarray(r["attn"]) for r in res.results], axis=0)
    return (ctx.astype(np.float32), attn.reshape(B, L, 1).astype(np.float32)), res


def kernel(**inputs):
    outs, _ = _run(inputs, trace=False)
    return outs
